# revision 1
# baseline (speedup 1.0000x reference)
"""Conformer block (macaron FF + RMLA attention + gated depthwise conv) on
8 Trainium2 NeuronCores, data-parallel over batch (B=8 -> 1 seq/core).

Design: the residual stream lives channel-major (transposed, [D, T]) in SBUF
for the whole kernel, so every matmul consumes/produces transposed
activations with natural-layout weights as the stationary operand and no
activation transposes are needed anywhere (attention softmax runs max-free
over the partition axis of transposed scores; denominators come free from a
ones-column appended to V). LayerNorm gammas/betas are folded into the
adjacent weights/biases on the host; each LN standardizes the stream once
into bf16 tiles (ones-matmul statistics + K=1 broadcast matmuls), so
matmul evicts are a single bias/activation op. The FFN first layers run
fp8e4m3 with DoubleRow perf mode (K=256 per PE pass, weights pre-scaled
x64, rescaled in the silu evict); everything else runs bf16. The depthwise
conv splits its 31 taps across PE diagonal matmuls (19), scalar-engine
per-partition-scale ACTs (8), and vector FMAs (4).
"""
import os
from contextlib import ExitStack

import numpy as np
import ml_dtypes

import concourse.bacc as bacc
import concourse.tile as tile
import concourse.mybir as mybir
from concourse.bass_utils import run_bass_kernel_spmd

B, T, D = 8, 1024, 1024
H, HD, KVH, R = 16, 64, 4, 256
KW = 31
FF = 4 * D
EPS = 1e-5
P = 128
DC = D // P            # 8 residual chunks
FFC = FF // P          # 32
RC = R // P            # 2
N_CORES = 8

dt = mybir.dt
Alu = mybir.AluOpType
Act = mybir.ActivationFunctionType

bf16 = ml_dtypes.bfloat16

PHASES = int(os.environ.get("BASS_PHASES", "5"))
DEBUG = int(os.environ.get("BASS_DEBUG", "0"))
MMDT = os.environ.get("BASS_MMDT", "bf16")

CSPLIT = int(os.environ.get("BASS_CSPLIT", "19"))  # conv taps on PE
CSC = int(os.environ.get("BASS_CSC", "8"))         # conv taps on scalar engine
FF8 = int(os.environ.get("BASS_FF8", "1"))         # fp8 DoubleRow FFN layer 1
F8SC = 64.0                                        # fp8 weight scale


# ---------------------------------------------------------------- host prep

def _shuffle_w(W):
    """[Kd, Nd] -> [NC, 128, Kd] where slab n is the SBUF lhsT tile for
    output chunk n: tile[:, kc*128:(kc+1)*128] = W[kc-chunk, n-chunk]."""
    Kd, Nd = W.shape
    KC, NC = Kd // P, Nd // P
    arr = W.reshape(KC, P, NC, P).transpose(2, 1, 0, 3).reshape(NC, P, Kd)
    return np.ascontiguousarray(arr)


def _cols(v):
    """[N] bias -> [128, N/128] column tile (col n = bias of chunk n)."""
    return np.ascontiguousarray(v.reshape(-1, P).T)


def prep_inputs(inputs):
    f32 = np.float32
    wdt = bf16 if MMDT == "bf16" else f32
    g = {}

    def W(name):
        return np.asarray(inputs[name], f32)

    def _w1(wf):
        """First-layer FFN weight slab: fp8 x64 in k-pair layout, or bf16."""
        if FF8:
            return np.ascontiguousarray(
                _shuffle_w(wf * F8SC).reshape(FFC, P, DC, P)).astype(
                    ml_dtypes.float8_e4m3)
        return _shuffle_w(wf).astype(bf16)

    # ff1 (LN gamma/beta folded; 0.5 residual scale folded into w2/b2)
    w1f = W('ff1_ng')[:, None] * W('ff1_w1')
    g['w1a'] = _w1(w1f)
    g['c1a'] = _cols(W('ff1_nb') @ W('ff1_w1') + W('ff1_b1'))
    g['w2a'] = _shuffle_w(0.5 * W('ff1_w2')).astype(bf16)
    g['c2a'] = _cols(0.5 * W('ff1_b2'))
    # attention projections
    wqf = W('attn_ng')[:, None] * W('wq')
    g['wqa'] = _shuffle_w(wqf).astype(bf16)
    g['cqa'] = _cols(W('attn_nb') @ W('wq'))
    wkvaf = (W('attn_ng')[:, None] * W('wkva'))[:, :R]
    g['wkvaa'] = _shuffle_w(wkvaf).astype(bf16)
    g['ckvaa'] = _cols((W('attn_nb') @ W('wkva'))[:R])
    g['wkvba'] = _shuffle_w(W('kvn_g')[:, None] * W('wkvb')).astype(wdt)
    g['ckvba'] = _cols(W('kvn_b') @ W('wkvb'))
    g['woa'] = _shuffle_w(W('wo')).astype(wdt)
    # conv module
    wp1f = W('conv_ng')[:, None] * W('pw1_w')
    g['wp1a'] = _shuffle_w(wp1f).astype(bf16)
    g['cp1a'] = _cols(W('conv_nb') @ W('pw1_w') + W('pw1_b'))
    sbn = W('bn_g') / np.sqrt(W('bn_rv') + EPS)
    g['sbna'] = _cols(sbn)
    g['tbna'] = _cols((W('dw_b') - W('bn_rm')) * sbn + W('bn_b'))
    g['wp2a'] = _shuffle_w(W('pw2_w')).astype(bf16)
    g['cp2a'] = _cols(W('pw2_b'))
    dwf = np.asarray(inputs['dw_w'], f32)[:, 0, :] * sbn[:, None]  # [D, 31]
    g['dwcol'] = np.ascontiguousarray(
        dwf.reshape(DC, P, KW).transpose(1, 0, 2).reshape(P, DC * KW))
    diag = np.zeros((DC, P, CSPLIT, P), f32)
    idx = np.arange(P)
    for c in range(DC):
        for j in range(CSPLIT):
            diag[c, idx, j, idx] = dwf[c * P:(c + 1) * P, j]
    g['diaga'] = diag.reshape(DC, P, CSPLIT * P).astype(bf16)
    # ff2
    w1bf = W('ff2_ng')[:, None] * W('ff2_w1')
    g['w1b'] = _w1(w1bf)
    g['c1b'] = _cols(W('ff2_nb') @ W('ff2_w1') + W('ff2_b1'))
    g['w2b'] = _shuffle_w(0.5 * W('ff2_w2')).astype(bf16)
    g['c2b'] = _cols(0.5 * W('ff2_b2'))
    # final LN affine
    g['finga'] = _cols(W('fin_g'))
    g['finba'] = _cols(W('fin_b'))
    # rope tables (transposed, tiled x2 heads per 128 partitions)
    inv = 1.0 / (10000.0 ** (np.arange(0, HD, 2, dtype=f32) / HD))
    t = np.arange(T, dtype=f32)
    fr = np.einsum('i,j->ij', t, inv)
    emb = np.concatenate([fr, fr], -1)                        # [T, 64]
    cosT = np.cos(emb).T.astype(f32)                          # [64, T]
    sinT = np.sin(emb).T.astype(f32)
    g['cos2'] = np.ascontiguousarray(
        np.concatenate([cosT, cosT], 0)).astype(bf16)
    g['sin2'] = np.ascontiguousarray(
        np.concatenate([sinT, sinT], 0)).astype(bf16)
    p2 = np.zeros((P, P), f32)
    for b in range(2):
        o = 64 * b
        for d_ in range(32):
            p2[o + 32 + d_, o + d_] = -1.0
            p2[o + d_, o + 32 + d_] = 1.0
    g['p2m'] = p2.astype(bf16)
    id2 = np.zeros((P, P), f32)
    id2[0:64, 0:64] = np.eye(64, dtype=f32)
    id2[64:P, 0:64] = np.eye(64, dtype=f32)
    g['ident'] = id2
    g['ones1'] = np.ones((1, P), f32)
    sel2 = np.zeros((2, P), f32)
    sel2[0, 0:64] = 1.0
    sel2[1, 64:P] = 1.0
    g['sel2'] = sel2
    g['onesp'] = np.ones((P, 1), f32)
    g['onespb'] = np.ones((P, 1), f32).astype(bf16)
    return g


# ------------------------------------------------------------- device build

def build():
    nc = bacc.Bacc("TRN2", target_bir_lowering=False, debug=False,
                   enable_asserts=False, num_devices=N_CORES)
    f32, f32r, b16 = dt.float32, dt.float32r, dt.bfloat16
    mmdt = b16 if MMDT == "bf16" else f32r

    def din(name, shape, d):
        return nc.dram_tensor(name, shape, d, kind="ExternalInput").ap()

    xT = din('xT', (D, T), f32r)
    f8 = dt.float8e4
    if FF8:
        w1a = din('w1a', (FFC, P, DC, P), f8)
    else:
        w1a = din('w1a', (FFC, P, D), b16)
    c1a = din('c1a', (P, FFC), f32)
    w2a = din('w2a', (DC, P, FF), b16)
    c2a = din('c2a', (P, DC), f32)
    wqa = din('wqa', (8, P, D), b16)
    cqa = din('cqa', (P, 8), f32)
    wkvaa = din('wkvaa', (RC, P, D), b16)
    ckvaa = din('ckvaa', (P, RC), f32)
    wkvba = din('wkvba', (4, P, R), mmdt)
    ckvba = din('ckvba', (P, 4), f32)
    woa = din('woa', (DC, P, D), mmdt)
    wp1a = din('wp1a', (16, P, D), b16)
    cp1a = din('cp1a', (P, 16), f32)
    sbna = din('sbna', (P, DC), f32)
    tbna = din('tbna', (P, DC), f32)
    wp2a = din('wp2a', (DC, P, D), b16)
    cp2a = din('cp2a', (P, DC), f32)
    dwcold = din('dwcol', (P, DC * KW), f32)
    diaga = din('diaga', (DC, P, CSPLIT * P), b16)
    if FF8:
        w1b = din('w1b', (FFC, P, DC, P), f8)
    else:
        w1b = din('w1b', (FFC, P, D), b16)
    c1b = din('c1b', (P, FFC), f32)
    w2b = din('w2b', (DC, P, FF), b16)
    c2b = din('c2b', (P, DC), f32)
    finga = din('finga', (P, DC), f32)
    finba = din('finba', (P, DC), f32)
    cos2d = din('cos2', (P, T), b16)
    sin2d = din('sin2', (P, T), b16)
    p2md = din('p2m', (P, P), b16)
    identd = din('ident', (P, P), f32r)
    ones1d = din('ones1', (1, P), f32r)
    sel2d = din('sel2', (2, P), f32r)
    onespd = din('onesp', (P, 1), f32r)
    onespbd = din('onespb', (P, 1), b16)

    outT = nc.dram_tensor('outT', (D, T), f32r, kind="ExternalOutput").ap()

    def ddram(name, shape, d):
        return nc.dram_tensor(name, shape, d, kind="ExternalOutput").ap()

    with tile.TileContext(nc) as tc, ExitStack() as top:
        cpool = top.enter_context(tc.tile_pool(name="const", bufs=1))
        res_pool = top.enter_context(tc.tile_pool(name="res", bufs=1))
        xh_pool = top.enter_context(tc.tile_pool(name="xh", bufs=1))

        res = []
        for c in range(DC):
            r_ = res_pool.tile([P, T], f32r, name=f"res{c}")
            nc.sync.dma_start(r_[:], xT[c * P:(c + 1) * P, :])
            res.append(r_)

        def ctile(src, shape, d, name):
            t_ = cpool.tile(shape, d, name=name)
            nc.sync.dma_start(t_[:], src[:])
            return t_

        c1t = ctile(c1a, [P, FFC], f32, "c1t")
        c2t = ctile(c2a, [P, DC], f32, "c2t")
        cqt = ctile(cqa, [P, 8], f32, "cqt")
        ckvat = ctile(ckvaa, [P, RC], f32, "ckvat")
        ckvbt = ctile(ckvba, [P, 4], f32, "ckvbt")
        cp1t = ctile(cp1a, [P, 16], f32, "cp1t")
        sbnt = ctile(sbna, [P, DC], f32, "sbnt")
        tbnt = ctile(tbna, [P, DC], f32, "tbnt")
        cp2t = ctile(cp2a, [P, DC], f32, "cp2t")
        c1bt = ctile(c1b, [P, FFC], f32, "c1bt")
        c2bt = ctile(c2b, [P, DC], f32, "c2bt")
        fingt = ctile(finga, [P, DC], f32, "fingt")
        finbt = ctile(finba, [P, DC], f32, "finbt")
        cos2t = ctile(cos2d, [P, T], b16, "cos2t")
        sin2t = ctile(sin2d, [P, T], b16, "sin2t")
        p2mt = ctile(p2md, [P, P], b16, "p2mt")
        identt = ctile(identd, [P, P], f32r, "identt")
        ones1t = ctile(ones1d, [1, P], f32r, "ones1t")
        sel2t = ctile(sel2d, [2, P], f32r, "sel2t")
        onespt = ctile(onespd, [P, 1], f32r, "onespt")
        onespbt = ctile(onespbd, [P, 1], b16, "onespbt")
        dwcolt = ctile(dwcold, [P, DC * KW], f32, "dwcolt")
        epst = cpool.tile([P, 1], dt.float32, name="epst")
        nc.gpsimd.memset(epst[:], EPS)

        def ln_stats(ctx, tag, src_tiles, nch, dred):
            """Stats of src over nch*128 channels; returns (A_b, NMA_b) SBUF
            [128,T] f32 broadcast tiles: xhat = src*A_b + NMA_b per token.
            PSUM/scratch pools are scoped to this call (freed on return)."""
            lnp = ctx.enter_context(
                tc.tile_pool(name=f"lnp_{tag}", bufs=2, space="PSUM"))
            lns = ctx.enter_context(tc.tile_pool(name=f"lns_{tag}", bufs=1))
            src_is_b16 = src_tiles[0].dtype == b16
            ones_s1 = onespbt if src_is_b16 else onespt

            def rd(ap):
                return ap[:] if src_is_b16 else ap.bitcast(f32)[:]

            sq = []
            for c in range(nch):
                s_ = lns.tile([P, T], b16, tag="sq", bufs=2,
                              name=f"sq_{tag}{c}")
                nc.scalar.square(s_[:], rd(src_tiles[c]))
                sq.append(s_)
            s1 = lnp.tile([1, T], f32, tag="lnps", name=f"s1_{tag}")
            s2 = lnp.tile([1, T], f32, tag="lnps", name=f"s2_{tag}")
            for c in range(nch):
                for h in range(2):
                    sl = slice(h * 512, (h + 1) * 512)
                    nc.tensor.matmul(s1[:, sl], ones_s1[:], src_tiles[c][:, sl],
                                     start=(c == 0), stop=(c == nch - 1))
            for c in range(nch):
                for h in range(2):
                    sl = slice(h * 512, (h + 1) * 512)
                    nc.tensor.matmul(s2[:, sl], onespbt[:], sq[c][:, sl],
                                     start=(c == 0), stop=(c == nch - 1))
            m_t = lns.tile([1, T], f32r, name=f"m_{tag}")
            a_t = lns.tile([1, T], f32r, name=f"a_{tag}")
            nc.vector.tensor_scalar(m_t[:], s1[:], 1.0 / dred, None, Alu.mult)
            ms = lns.tile([1, T], f32, name=f"ms_{tag}")
            nc.scalar.square(ms[:], m_t.bitcast(f32)[:])
            v_ = lns.tile([1, T], f32, name=f"v_{tag}")
            nc.vector.scalar_tensor_tensor(v_[:], s2[:], 1.0 / dred, ms[:],
                                           Alu.mult, Alu.subtract)
            sd = lns.tile([1, T], f32, name=f"sd_{tag}")
            nc.scalar.activation(sd[:], v_[:], Act.Sqrt, bias=epst[0:1, 0:1])
            af = lns.tile([1, T], f32, name=f"af_{tag}")
            nc.vector.reciprocal_approx_fast(out=af[:], in_=sd[:])
            nc.vector.tensor_copy(a_t[:], af[:])
            nma_t = lns.tile([1, T], f32r, name=f"nma_{tag}")
            nc.vector.scalar_tensor_tensor(nma_t[:], m_t.bitcast(f32)[:],
                                           -1.0, af[:], Alu.mult, Alu.mult)
            abp = lnp.tile([P, T], f32, tag="lnps", name=f"abp_{tag}")
            nmp = lnp.tile([P, T], f32, tag="lnps", name=f"nmp_{tag}")
            for h in range(2):
                sl = slice(h * 512, (h + 1) * 512)
                nc.tensor.matmul(abp[:, sl], ones1t[:], a_t[:, sl],
                                 start=True, stop=True)
                nc.tensor.matmul(nmp[:, sl], ones1t[:], nma_t[:, sl],
                                 start=True, stop=True)
            ab_s = xh_pool.tile([P, T], f32, tag="stA", name=f"abs_{tag}")
            nmb_s = xh_pool.tile([P, T], f32, tag="stM", name=f"nmbs_{tag}")
            nc.scalar.copy(ab_s[:], abp[:])
            nc.scalar.copy(nmb_s[:], nmp[:])
            return ab_s, nmb_s

        def ln_std(ctx, tag, src_tiles, nch, dred, xh_tag="xh"):
            """Standardized bf16 copies of src (over nch*128 channels).
            All scratch (incl. psum) is freed before returning."""
            del ctx
            with ExitStack() as ictx:
                ab_s, nmb_s = ln_stats(ictx, tag, src_tiles, nch, dred)
                lnu = ictx.enter_context(
                    tc.tile_pool(name=f"lnu_{tag}", bufs=1))
                src_is_b16 = src_tiles[0].dtype == b16
                outs = []
                for c in range(nch):
                    tm = lnu.tile([P, T], f32, tag="lnt", bufs=1,
                                  name=f"lnt_{tag}{c}")
                    src_r = (src_tiles[c][:] if src_is_b16
                             else src_tiles[c].bitcast(f32)[:])
                    nc.vector.tensor_tensor(tm[:], src_r, ab_s[:], Alu.mult)
                    t_ = xh_pool.tile([P, T], b16, tag=f"{xh_tag}{c}",
                                      name=f"xhs_{tag}{c}")
                    nc.vector.tensor_tensor(t_[:], tm[:], nmb_s[:], Alu.add)
                    outs.append(t_)
            return outs

        def mmproj(pool, wt, rhs, kc, nm, evict, kslice=None):
            """out[:,h*512:(h+1)*512] = sum_k wt[:,k].T @ rhs[k][:,h*512:...];
            k-outer so each stationary is loaded once for both halves."""
            ps = [pool.tile([P, 512], dt.float32, tag="mm",
                            name=f"{nm}_h{h}") for h in range(2)]
            for k in range(kc):
                w_ = wt[:, k * P:(k + 1) * P]
                r_ = rhs[k]
                for h in range(2):
                    nc.tensor.matmul(ps[h][:], w_,
                                     r_[:, h * 512:(h + 1) * 512],
                                     start=(k == 0), stop=(k == kc - 1))
            for h in range(2):
                evict(h, ps[h])

        # ---------------- feed-forward macaron ----------------
        def ffn(tag, w1d, c1tile, w2d, c2tile, after_dch=None):
            with ExitStack() as ctx:
                xh = ln_std(ctx, tag, res, DC, D)
                wp = ctx.enter_context(tc.tile_pool(name=f"w_{tag}", bufs=3))
                hp = ctx.enter_context(tc.tile_pool(name=f"h1_{tag}", bufs=1))
                pp = ctx.enter_context(
                    tc.tile_pool(name=f"ps_{tag}", bufs=4, space="PSUM"))
                if FF8:
                    # pack standardized activations as fp8 k-pair tiles for
                    # DoubleRow (contraction 256 per pass)
                    xq = []
                    for j in range(DC // 2):
                        t_ = hp.tile([P, 2, T], dt.float8e4, tag=f"xq{j}",
                                     name=f"xq_{tag}{j}")
                        nc.vector.tensor_copy(t_[:, 0, :], xh[2 * j][:])
                        nc.vector.tensor_copy(t_[:, 1, :], xh[2 * j + 1][:])
                        xq.append(t_)
                h1 = []
                for n in range(FFC):
                    h_ = hp.tile([P, T], b16, tag=f"h1_{n}", name=f"h1_{tag}{n}")
                    if FF8:
                        wt = wp.tile([P, DC, P], dt.float8e4, tag="w1",
                                     name=f"w1_{tag}{n}")
                        nc.sync.dma_start(wt[:], w1d[n])
                        psh = [pp.tile([P, 512], f32, tag="mm",
                                       name=f"p1_{tag}{n}_h{h}")
                               for h in range(2)]
                        for k in range(DC // 2):
                            for h in range(2):
                                nc.tensor.matmul(
                                    psh[h][:], wt[:, 2 * k:2 * k + 2, :],
                                    xq[k][:, :, h * 512:(h + 1) * 512],
                                    start=(k == 0), stop=(k == DC // 2 - 1),
                                    perf_mode=mybir.MatmulPerfMode.DoubleRow)
                        for h in range(2):
                            nc.scalar.activation(
                                h_[:, h * 512:(h + 1) * 512], psh[h][:],
                                Act.Silu, bias=c1tile[:, n:n + 1],
                                scale=1.0 / F8SC)
                    else:
                        wt = wp.tile([P, D], b16, tag="w1",
                                     name=f"w1_{tag}{n}")
                        nc.sync.dma_start(wt[:], w1d[n])

                        def ev1(h, ps, h_=h_, n=n):
                            sl = slice(h * 512, (h + 1) * 512)
                            nc.scalar.activation(h_[:, sl], ps[:], Act.Silu,
                                                 bias=c1tile[:, n:n + 1])
                        mmproj(pp, wt, xh, DC, f"p1_{tag}{n}", ev1)
                    h1.append(h_)
                if DEBUG and tag == "ff1":
                    nc.sync.dma_start(ddram('d_h1', (P, T), b16)[:], h1[0][:])
                for dch in range(DC):
                    wt = wp.tile([P, FF], b16, tag="w2", bufs=2,
                                 name=f"w2_{tag}{dch}")
                    nc.sync.dma_start(wt[:], w2d[dch])
                    def ev2(h, ps, dch=dch):
                        sl = slice(h * 512, (h + 1) * 512)
                        nc.vector.scalar_tensor_tensor(
                            res[dch][:, sl], ps[:], c2tile[:, dch:dch + 1],
                            res[dch].bitcast(f32)[:, sl], Alu.add, Alu.add)
                    mmproj(pp, wt, h1, FFC, f"p2_{tag}{dch}", ev2)
                    if after_dch is not None and dch > 0:
                        after_dch(dch - 1)
                if after_dch is not None:
                    after_dch(DC - 1)

        # ---------------- layernorm (standardize only) ----------------
        def ln(tag, src_tiles, nch, dred):
            """Standardize src over nch*128 channels; out tiles (mmdt)."""
            src_is_b16 = src_tiles[0].dtype == b16
            ones_stat = onespbt if src_is_b16 else onespt

            def rd(ap):
                return ap if src_is_b16 else ap.bitcast(f32)

            with ExitStack() as ctx:
                lnp = ctx.enter_context(
                    tc.tile_pool(name=f"lnp_{tag}", bufs=2, space="PSUM"))
                lns = ctx.enter_context(tc.tile_pool(name=f"lns_{tag}", bufs=1))
                sq = []
                for c in range(nch):
                    s_ = lns.tile([P, T], b16, tag="sq", bufs=2,
                                  name=f"sq_{tag}{c}")
                    nc.scalar.square(s_[:], rd(src_tiles[c][:]))
                    sq.append(s_)
                s1 = lnp.tile([1, T], f32, tag="lnps", name=f"s1_{tag}")
                s2 = lnp.tile([1, T], f32, tag="lnps", name=f"s2_{tag}")
                for c in range(nch):
                    for h in range(2):
                        sl = slice(h * 512, (h + 1) * 512)
                        nc.tensor.matmul(s1[:, sl], ones_stat[:],
                                         src_tiles[c][:, sl],
                                         start=(c == 0), stop=(c == nch - 1))
                for c in range(nch):
                    for h in range(2):
                        sl = slice(h * 512, (h + 1) * 512)
                        nc.tensor.matmul(s2[:, sl], onespbt[:], sq[c][:, sl],
                                         start=(c == 0), stop=(c == nch - 1))
                m_t = lns.tile([1, T], f32r, name=f"m_{tag}")
                a_t = lns.tile([1, T], f32r, name=f"a_{tag}")
                m_r = m_t[:]
                a_r = a_t[:]
                nc.vector.tensor_scalar(m_r, s1[:], 1.0 / dred, None, Alu.mult)
                ms = lns.tile([1, T], f32, name=f"ms_{tag}")
                nc.scalar.square(ms[:], m_r.bitcast(f32))
                v_ = lns.tile([1, T], f32, name=f"v_{tag}")
                nc.vector.scalar_tensor_tensor(v_[:], s2[:], 1.0 / dred, ms[:],
                                               Alu.mult, Alu.subtract)
                sd = lns.tile([1, T], f32, name=f"sd_{tag}")
                nc.scalar.activation(sd[:], v_[:], Act.Sqrt, bias=epst[0:1, 0:1])
                af = lns.tile([1, T], f32, name=f"af_{tag}")
                nc.vector.reciprocal_approx_fast(out=af[:], in_=sd[:])
                nc.vector.tensor_copy(a_r, af[:])
                mb = lnp.tile([P, T], f32, tag="lnps", name=f"mb_{tag}")
                ab = lnp.tile([P, T], f32, tag="lnps", name=f"ab_{tag}")
                for h in range(2):
                    sl = slice(h * 512, (h + 1) * 512)
                    nc.tensor.matmul(mb[:, sl], ones1t[:], m_r[:, sl],
                                     start=True, stop=True)
                    nc.tensor.matmul(ab[:, sl], ones1t[:], a_r[:, sl],
                                     start=True, stop=True)
                outs = []
                for c in range(nch):
                    t_ = xh_pool.tile([P, T], mmdt, tag=f"xh{c}",
                                      name=f"xh_{tag}{c}")
                    tm = lns.tile([P, T], f32, tag="lntmp", bufs=2,
                                  name=f"lntmp_{tag}{c}")
                    for h in range(2):
                        sl = slice(h * 512, (h + 1) * 512)
                        nc.vector.tensor_tensor(tm[:, sl],
                                                rd(src_tiles[c][:, sl]),
                                                mb[:, sl], Alu.subtract)
                        nc.vector.tensor_tensor(t_[:, sl], tm[:, sl],
                                                ab[:, sl], Alu.mult)
                    outs.append(t_)
            return outs

        # ---------------- attention ----------------
        def attn():
            with ExitStack() as ctx:
                wp = ctx.enter_context(tc.tile_pool(name="w_at", bufs=2))
                kv_pool = ctx.enter_context(tc.tile_pool(name="kvt", bufs=1))
                fv = ctx.enter_context(tc.tile_pool(name="fv_at", bufs=4))

                qpre, kva = [], []
                with tc.tile_pool(name="pA", bufs=4, space="PSUM") as pA:
                    with ExitStack() as lctx:
                        xh = ln_std(lctx, "at", res, DC, D)

                        # kva projection first: the kv-latent LN chain
                        # then overlaps the q-projection PE bursts
                        for n in range(RC):
                            wt = wp.tile([P, D], b16, tag="w1",
                                         name=f"wkva{n}")
                            nc.sync.dma_start(wt[:], wkvaa[n])
                            kv_ = kv_pool.tile([P, T], mmdt, tag=f"kva{n}",
                                               name=f"kva{n}")

                            def evkva(h, ps, kv_=kv_, n=n):
                                sl = slice(h * 512, (h + 1) * 512)
                                nc.vector.tensor_scalar(
                                    kv_[:, sl], ps[:], ckvat[:, n:n + 1],
                                    None, Alu.add)
                            mmproj(pA, wt, xh, DC, f"pkva{n}", evkva)
                            kva.append(kv_)
                        # q projection -> qpre (bf16, pre-rope)
                        for n in range(8):
                            wt = wp.tile([P, D], b16, tag="w1",
                                         name=f"wq{n}")
                            nc.sync.dma_start(wt[:], wqa[n])
                            q_ = kv_pool.tile([P, T], b16, tag=f"q{n}",
                                              name=f"qpre{n}")

                            def evq(h, ps, q_=q_, n=n):
                                sl = slice(h * 512, (h + 1) * 512)
                                nc.vector.tensor_scalar(
                                    q_[:, sl], ps[:], cqt[:, n:n + 1],
                                    None, Alu.add)
                            mmproj(pA, wt, xh, DC, f"pq{n}", evq)
                            qpre.append(q_)
                    # latent LN (own psum pool: 4 + 4 banks ok)
                    lat = ln("kv", kva, RC, R)
                    # kvb projection: kv rows 0..255 = k, 256..511 = v
                    kpre, vtt = [], []
                    for n in range(4):
                        wt = wp.tile([P, R], mmdt, tag="wkvb", name=f"wkvb{n}")
                        nc.sync.dma_start(wt[:], wkvba[n])
                        kv_ = kv_pool.tile([P, T], b16 if n < 2 else f32r,
                                           tag=f"kvb{n}", name=f"kvb{n}")
                        mmproj(pA, wt, lat, RC, f"pkvb{n}",
                               lambda h, ps, kv_=kv_, n=n:
                               nc.scalar.activation(
                                   kv_[:, h * 512:(h + 1) * 512], ps[:],
                                   Act.Identity, bias=ckvbt[:, n:n + 1]))
                        (kpre if n < 2 else vtt).append(kv_)
                    # v: transpose to token-major + ones col -> v_aug (bf16)
                    vaug = []
                    for c in range(DC):
                        va = kv_pool.tile([P, KVH * 65], b16, tag=f"va{c}",
                                          name=f"vaug{c}")
                        for g_ in range(KVH):
                            nc.gpsimd.memset(
                                va[:, g_ * 65 + 64:g_ * 65 + 65], 1.0)
                        vaug.append(va)
                    for g_ in range(KVH):
                        src = vtt[g_ // 2]
                        off = 64 * (g_ % 2)
                        for c in range(DC):
                            pt_ = pA.tile([P, 64], f32r, tag="mm",
                                          name=f"vt{g_}_{c}")
                            nc.tensor.matmul(pt_[:],
                                             src[off:off + 64,
                                                 c * P:(c + 1) * P],
                                             identt[off:off + 64, 0:64],
                                             is_transpose=True,
                                             start=True, stop=True)
                            nc.scalar.copy(
                                vaug[c][:, g_ * 65:g_ * 65 + 64],
                                pt_.bitcast(f32)[:])

                # rope on q and k -> bf16
                roped = []
                with tc.tile_pool(name="pR", bufs=2, space="PSUM") as pR:
                    for i, src in enumerate(qpre + kpre):
                        is_q = i < 8
                        pq = pR.tile([P, T], f32, tag="rope", name=f"ropep{i}")
                        for h in range(2):
                            sl = slice(h * 512, (h + 1) * 512)
                            nc.tensor.matmul(pq[:, sl], p2mt[:], src[:, sl],
                                             start=True, stop=True)
                        pqs = kv_pool.tile([P, T], b16, tag="pqs", bufs=2,
                                           name=f"pqs{i}")
                        nc.scalar.copy(pqs[:], pq[:])
                        t1 = kv_pool.tile([P, T], b16, tag="ropet1", bufs=2,
                                          name=f"ropet1_{i}")
                        nc.vector.tensor_tensor(t1[:], src[:], cos2t[:],
                                                Alu.mult)
                        t2 = kv_pool.tile([P, T], b16, tag="ropet2", bufs=2,
                                          name=f"ropet2_{i}")
                        nc.vector.tensor_tensor(t2[:], pqs[:], sin2t[:],
                                                Alu.mult)
                        r_ = kv_pool.tile(
                            [P, T], b16,
                            tag=(f"q{i}" if is_q else f"kro{i}"),
                            name=f"roped{i}")
                        nc.vector.tensor_tensor(r_[:], t1[:], t2[:], Alu.add)
                        roped.append(r_)
                qr, krc = roped[:8], roped[8:]
                kr2 = []
                for g_ in range(KVH):
                    k2 = kv_pool.tile([P, T], b16, tag=f"kr2_{g_}",
                                      name=f"kr2_{g_}")
                    off = 64 * (g_ % 2)
                    src = krc[g_ // 2]
                    nc.vector.tensor_copy(k2[0:64, :], src[off:off + 64, :])
                    nc.vector.tensor_copy(k2[64:P, :], src[off:off + 64, :])
                    kr2.append(k2)

                # scores -> exp -> pT ; oT via v_aug (denominator in row 64);
                # per-head-pair softmax normalization
                ots = []
                with ExitStack() as sctx:
                    scp = sctx.enter_context(
                        tc.tile_pool(name="scp", bufs=4, space="PSUM"))
                    otp = sctx.enter_context(
                        tc.tile_pool(name="otp", bufs=2, space="PSUM"))
                    rbp = sctx.enter_context(
                        tc.tile_pool(name="rbp", bufs=1, space="PSUM"))
                    ptp = sctx.enter_context(tc.tile_pool(name="ptp", bufs=2))
                    otup = sctx.enter_context(tc.tile_pool(name="otup", bufs=1))
                    for hp in range(8):
                        g_ = (2 * hp) // 4
                        kt = kr2[g_]
                        ptsub = []
                        for sub in range(2):
                            hh = 2 * hp + sub
                            ptsub.append([ptp.tile([P, T], b16, tag=f"pt{c}",
                                                   name=f"pt{hh}_{c}")
                                          for c in range(DC)])
                        for c in range(DC):
                            for th in range(2):
                                sl = slice(th * 512, (th + 1) * 512)
                                pss = []
                                for sub in range(2):
                                    hh = 2 * hp + sub
                                    qt, qo = qr[hh // 2], 64 * sub
                                    ps = scp.tile([P, 512], f32, tag="sc",
                                                  name=f"sc{hh}_{c}_{th}")
                                    nc.tensor.matmul(
                                        ps[:],
                                        kt[qo:qo + 64, c * P:(c + 1) * P],
                                        qt[qo:qo + 64, sl],
                                        start=True, stop=True)
                                    pss.append(ps)
                                for sub in range(2):
                                    nc.scalar.activation(
                                        ptsub[sub][c][:, sl], pss[sub][:],
                                        Act.Exp, scale=float(HD) ** -0.5)
                        otu2 = []
                        for sub in range(2):
                            hh = 2 * hp + sub
                            pts = ptsub[sub]
                            ou = otup.tile([65, T], f32, tag=f"otu{sub}",
                                           bufs=2, name=f"otu{hh}")
                            pos = [otp.tile([65, 512], f32, tag="ot",
                                            name=f"ot{hh}_{th}")
                                   for th in range(2)]
                            for c in range(DC):
                                for th in range(2):
                                    sl = slice(th * 512, (th + 1) * 512)
                                    nc.tensor.matmul(
                                        pos[th][:],
                                        vaug[c][:, g_ * 65:(g_ + 1) * 65],
                                        pts[c][:, sl],
                                        start=(c == 0), stop=(c == DC - 1))
                            for th in range(2):
                                sl = slice(th * 512, (th + 1) * 512)
                                nc.vector.tensor_copy(ou[:, sl], pos[th][:])
                            otu2.append(ou)
                        # pair normalize
                        den2 = otup.tile([2, T], f32, tag="den", bufs=1,
                                         name=f"den{hp}")
                        for sub in range(2):
                            nc.sync.dma_start(den2[sub:sub + 1, :],
                                              otu2[sub][64:65, :])
                        recf2 = otup.tile([2, T], f32, tag="recf", bufs=1,
                                          name=f"recf{hp}")
                        nc.vector.reciprocal_approx_fast(out=recf2[:],
                                                         in_=den2[:])
                        recr2 = otup.tile([2, T], f32r, tag="recr",
                                          bufs=1, name=f"recr{hp}")
                        nc.vector.tensor_copy(recr2[:], recf2[:])
                        rb = rbp.tile([P, T], f32, tag="rb", name=f"rb{hp}")
                        for th in range(2):
                            sl = slice(th * 512, (th + 1) * 512)
                            nc.tensor.matmul(rb[:, sl], sel2t[:],
                                             recr2[:, sl],
                                             start=True, stop=True)
                        o_ = kv_pool.tile([P, T], mmdt, tag=f"ot{hp}",
                                          name=f"ots{hp}")
                        for sub in range(2):
                            nc.vector.tensor_tensor(
                                o_[sub * 64:(sub + 1) * 64, :],
                                otu2[sub][0:64, :],
                                rb[sub * 64:(sub + 1) * 64, :], Alu.mult)
                        ots.append(o_)
                # output projection + residual
                with tc.tile_pool(name="pO", bufs=6, space="PSUM") as pO:
                    for dch in range(DC):
                        wt = wp.tile([P, D], mmdt, tag="w1", name=f"wo{dch}")
                        nc.sync.dma_start(wt[:], woa[dch])
                        def evo(h, ps, dch=dch):
                            sl = slice(h * 512, (h + 1) * 512)
                            nc.vector.tensor_tensor(
                                res[dch][:, sl], ps[:],
                                res[dch].bitcast(f32)[:, sl], Alu.add)
                        mmproj(pO, wt, ots, DC, f"po{dch}", evo)
        # ---------------- conv module ----------------
        def convmod():
            with ExitStack() as ctx:
                xh = ln_std(ctx, "cv", res, DC, D)
                wp = ctx.enter_context(tc.tile_pool(name="w_cv", bufs=3))
                ap_ = ctx.enter_context(tc.tile_pool(name="a_cv", bufs=1))
                pp = ctx.enter_context(
                    tc.tile_pool(name="ps_cv", bufs=6, space="PSUM"))
                # pw1 "a" half lands directly in the padded glu tiles
                # (16 zeros left/right); sigmoid half in sg tiles; then GLU
                # is an in-place multiply. glus = glu shifted by one element
                # so odd-offset bf16 taps stay 4B-aligned for DVE 2x.
                glu, sg = [], []
                for c in range(DC):
                    gp = ap_.tile([P, T + 32], b16, tag=f"glu{c}",
                                  name=f"glu{c}")
                    nc.gpsimd.memset(gp[:, 0:16], 0.0)
                    nc.gpsimd.memset(gp[:, T + 16:T + 32], 0.0)
                    glu.append(gp)
                for n in range(16):
                    wt = wp.tile([P, D], b16, tag="w1", name=f"wp1_{n}")
                    nc.sync.dma_start(wt[:], wp1a[n])
                    if n < 8:
                        o_ = glu[n]

                        def evc(h, ps, o_=o_, n=n):
                            sl = slice(16 + h * 512, 16 + (h + 1) * 512)
                            nc.scalar.activation(o_[:, sl], ps[:],
                                                 Act.Identity,
                                                 bias=cp1t[:, n:n + 1])
                    else:
                        o_ = ap_.tile([P, T], b16, tag=f"sg{n}", name=f"sg{n}")
                        sg.append(o_)

                        def evc(h, ps, o_=o_, n=n):
                            sl = slice(h * 512, (h + 1) * 512)
                            nc.scalar.activation(o_[:, sl], ps[:],
                                                 Act.Sigmoid,
                                                 bias=cp1t[:, n:n + 1])
                    mmproj(pp, wt, xh, DC, f"pp1_{n}", evc)
                for c in range(DC):
                    nc.vector.tensor_tensor(glu[c][:, 16:T + 16],
                                            glu[c][:, 16:T + 16], sg[c][:],
                                            Alu.mult)
                if DEBUG:
                    nc.sync.dma_start(ddram('d_glu', (P, T + 32), b16)[:],
                                      glu[0][:])
                # conv taps: out[t] += dwf[:, j] * pad[t + j + 1]
                cv = []
                for c in range(DC):
                    wt = wp.tile([P, CSPLIT * P], b16, tag="diag", bufs=2,
                                 name=f"dg{c}")
                    nc.sync.dma_start(wt[:], diaga[c])

                    def tapsrc(j, c=c):
                        return glu[c][:, j + 1:j + 1 + T]

                    def wcol(j, c=c):
                        return dwcolt[:, c * KW + j:c * KW + j + 1]

                    # scalar-engine taps: per-channel multiply via ACT scale
                    tmps = []
                    for j in range(CSPLIT, CSPLIT + CSC):
                        tm_ = ap_.tile([P, T], b16, tag="ctmp", bufs=CSC + 2,
                                       name=f"ctmp{c}_{j}")
                        nc.scalar.activation(tm_[:], tapsrc(j), Act.Identity,
                                             scale=wcol(j))
                        tmps.append(tm_)
                    accv = ap_.tile([P, T], b16, tag="caccv", bufs=2,
                                    name=f"caccv{c}")
                    j0 = CSPLIT + CSC
                    nc.vector.tensor_scalar(accv[:], tapsrc(j0), wcol(j0),
                                            None, Alu.mult)
                    for j in range(j0 + 1, KW):
                        nc.vector.scalar_tensor_tensor(
                            accv[:], tapsrc(j), wcol(j), accv[:],
                            Alu.mult, Alu.add)
                    for tm_ in tmps:
                        nc.vector.tensor_tensor(accv[:], accv[:], tm_[:],
                                                Alu.add)
                    psc = [pp.tile([P, 512], f32, tag="mm",
                                   name=f"pcv{c}_{th}") for th in range(2)]
                    for j in range(CSPLIT):
                        for th in range(2):
                            o = th * 512 + j + 1
                            nc.tensor.matmul(
                                psc[th][:], wt[:, j * P:(j + 1) * P],
                                glu[c][:, o:o + 512],
                                start=(j == 0), stop=(j == CSPLIT - 1))
                    o_ = ap_.tile([P, T], b16, tag=f"cv{c}", name=f"cv{c}")
                    for th in range(2):
                        sl = slice(th * 512, (th + 1) * 512)
                        z_ = ap_.tile([P, 512], f32, tag="cz", bufs=2,
                                      name=f"cz{c}_{th}")
                        nc.vector.tensor_tensor(z_[:], accv[:, sl], psc[th][:],
                                                Alu.add)
                        nc.scalar.activation(o_[:, sl], z_[:], Act.Silu,
                                             bias=tbnt[:, c:c + 1])
                    cv.append(o_)
                if DEBUG:
                    nc.sync.dma_start(ddram('d_cv', (P, T), b16)[:], cv[0][:])
                for dch in range(DC):
                    wt = wp.tile([P, D], b16, tag="wp2", bufs=2, name=f"wp2_{dch}")
                    nc.sync.dma_start(wt[:], wp2a[dch])
                    def evp2(h, ps, dch=dch):
                        sl = slice(h * 512, (h + 1) * 512)
                        nc.vector.scalar_tensor_tensor(
                            res[dch][:, sl], ps[:], cp2t[:, dch:dch + 1],
                            res[dch].bitcast(f32)[:, sl], Alu.add, Alu.add)
                    mmproj(pp, wt, cv, DC, f"pp2_{dch}", evp2)

        # ---------------- final LN (with affine) ----------------
        # squares + s1/s2 accumulation are emitted per-chunk (one chunk
        # lagged) inside ff2's L2 loop; finish() runs the small chain and
        # the normalize/store sweep.
        def make_final(ctx):
            lnp = ctx.enter_context(
                tc.tile_pool(name="lnp_fin", bufs=2, space="PSUM"))
            lns = ctx.enter_context(tc.tile_pool(name="lns_fin", bufs=1))
            s1 = lnp.tile([1, T], f32, tag="lnps", name="s1_fin")
            s2 = lnp.tile([1, T], f32, tag="lnps", name="s2_fin")

            def after_dch(dch):
                s_ = xh_pool.tile([P, T], b16, tag=f"xh{dch}",
                                  name=f"sq_fin{dch}")
                nc.scalar.square(s_[:], res[dch].bitcast(f32))
                for h in range(2):
                    sl = slice(h * 512, (h + 1) * 512)
                    nc.tensor.matmul(s1[:, sl], onespt[:], res[dch][:, sl],
                                     start=(dch == 0), stop=(dch == DC - 1))
                    nc.tensor.matmul(s2[:, sl], onespbt[:], s_[:, sl],
                                     start=(dch == 0), stop=(dch == DC - 1))

            def finish():
                with ExitStack() as fctx:
                    outp = fctx.enter_context(
                        tc.tile_pool(name="outp", bufs=2))
                    m_t = lns.tile([1, T], f32r, name="m_fin")
                    a_t = lns.tile([1, T], f32r, name="a_fin")
                    m_r, a_r = m_t[:], a_t[:]
                    nc.vector.tensor_scalar(m_r, s1[:], 1.0 / D, None,
                                            Alu.mult)
                    ms = lns.tile([1, T], f32, name="ms_fin")
                    nc.scalar.square(ms[:], m_r.bitcast(f32))
                    v_ = lns.tile([1, T], f32, name="v_fin")
                    nc.vector.scalar_tensor_tensor(v_[:], s2[:], 1.0 / D,
                                                   ms[:], Alu.mult,
                                                   Alu.subtract)
                    sd = lns.tile([1, T], f32, name="sd_fin")
                    nc.scalar.activation(sd[:], v_[:], Act.Sqrt,
                                         bias=epst[0:1, 0:1])
                    af = lns.tile([1, T], f32, name="af_fin")
                    nc.vector.reciprocal_approx_fast(out=af[:], in_=sd[:])
                    nc.vector.tensor_copy(a_r, af[:])
                    mb = lnp.tile([P, T], f32, tag="lnps", name="mb_fin")
                    ab = lnp.tile([P, T], f32, tag="lnps", name="ab_fin")
                    for h in range(2):
                        sl = slice(h * 512, (h + 1) * 512)
                        nc.tensor.matmul(mb[:, sl], ones1t[:], m_r[:, sl],
                                         start=True, stop=True)
                        nc.tensor.matmul(ab[:, sl], ones1t[:], a_r[:, sl],
                                         start=True, stop=True)
                    for c in range(DC):
                        tm = lns.tile([P, T], f32, tag="lntmp", bufs=2,
                                      name=f"fintmp{c}")
                        nc.vector.tensor_tensor(tm[:], res[c].bitcast(f32),
                                                mb[:], Alu.subtract)
                        u_ = lns.tile([P, T], f32, tag="lnu", bufs=2,
                                      name=f"finu{c}")
                        nc.vector.scalar_tensor_tensor(
                            u_[:], tm[:], fingt[:, c:c + 1], ab[:],
                            Alu.mult, Alu.mult)
                        o_ = outp.tile([P, T], f32r, tag="out",
                                       name=f"out{c}")
                        nc.scalar.activation(o_[:], u_[:], Act.Identity,
                                             bias=finbt[:, c:c + 1])
                        nc.sync.dma_start(outT[c * P:(c + 1) * P, :], o_[:])
            return after_dch, finish

        # ---------------- phase sequencing ----------------
        ffn("ff1", w1a, c1t, w2a, c2t)
        if DEBUG:
            dr1 = ddram('d_res1', (D, T), f32r)
            for c in range(DC):
                nc.sync.dma_start(dr1[c * P:(c + 1) * P, :], res[c][:])
        if PHASES >= 2:
            attn()
            if DEBUG:
                dr2 = ddram('d_res2', (D, T), f32r)
                for c in range(DC):
                    nc.sync.dma_start(dr2[c * P:(c + 1) * P, :], res[c][:])
        if PHASES >= 3:
            convmod()
            if DEBUG:
                dr3 = ddram('d_res3', (D, T), f32r)
                for c in range(DC):
                    nc.sync.dma_start(dr3[c * P:(c + 1) * P, :], res[c][:])
        if PHASES >= 5:
            with ExitStack() as fin_ctx:
                after_dch, fin = make_final(fin_ctx)
                ffn("ff2", w1b, c1bt, w2b, c2bt, after_dch=after_dch)
                fin()
        elif PHASES >= 4:
            ffn("ff2", w1b, c1bt, w2b, c2bt)
        if PHASES < 5:
            for c in range(DC):
                nc.sync.dma_start(outT[c * P:(c + 1) * P, :], res[c][:])

    nc.compile()
    return nc


# ------------------------------------------------------------------ driver

_NC_CACHE = {}


def _get_nc():
    key = (PHASES, DEBUG)
    if key not in _NC_CACHE:
        _NC_CACHE[key] = build()
    return _NC_CACHE[key]


def kernel(**inputs):
    nc = _get_nc()
    shared = prep_inputs(inputs)
    x = np.asarray(inputs['x'], np.float32)
    in_maps = []
    for b in range(N_CORES):
        m = dict(shared)
        m['xT'] = np.ascontiguousarray(x[b].T)
        in_maps.append(m)
    res = run_bass_kernel_spmd(nc, in_maps, core_ids=list(range(N_CORES)))
    out = np.stack([np.ascontiguousarray(r['outT'].T) for r in res.results])
    kernel.last_results = res
    return out.astype(np.float32)



# revision 13
# speedup vs baseline: 1.1594x; 1.1594x over previous
"""Conformer block (macaron FF + RMLA attention + gated depthwise conv) on
8 Trainium2 NeuronCores, data-parallel over batch (B=8 -> 1 seq/core).

Design: the residual stream lives channel-major ([D, T]) in SBUF for the
whole kernel; every matmul consumes/produces transposed activations with
natural-layout weights as the stationary operand. All large projections
(FFN both layers, q/kva, o, pw1, pw2) run fp8e4m3 with DoubleRow perf
mode (K=256 per PE pass, weights pre-scaled x64, rescaled in the evict);
activations are quantized to fp8 pair-packed tiles directly by the
standardize/evict ops that produce them. LayerNorm stats (squares +
ones-matmul sums) are emitted per-chunk inside the PREVIOUS phase's
output loop so only the short scalar chain sits on the phase boundary;
stat sums for one LN live in a single [33,T] PSUM tile (s1 row 0, s2 row
32 - col-packed ones-matmuls). Softmax: scores stay bf16 (row-packed
K=64 pairs on the PE); exp is split between the scalar engine (ACT Exp
on [128,1024] PSUM pairs) and the vector engine (Schraudolph bit-trick:
int16 = s*2^7/ln2 + b, bitcast to bf16). The denominator comes free from
a ones-column appended to V. The depthwise conv splits its 31 taps
across PE diagonal matmuls, scalar-engine per-partition-scale ACTs, and
vector FMAs, with the vector accumulator folded into the tap PSUM pair.
"""
import os
from contextlib import ExitStack

import numpy as np
import ml_dtypes

import concourse.bacc as bacc
import concourse.tile as tile
import concourse.mybir as mybir
from concourse.bass_utils import run_bass_kernel_spmd

B, T, D = 8, 1024, 1024
H, HD, KVH, R = 16, 64, 4, 256
KW = 31
FF = 4 * D
EPS = 1e-5
P = 128
DC = D // P            # 8 residual chunks
FFC = FF // P          # 32
RC = R // P            # 2
N_CORES = 8

dt = mybir.dt
Alu = mybir.AluOpType
Act = mybir.ActivationFunctionType
DR = mybir.MatmulPerfMode.DoubleRow

bf16 = ml_dtypes.bfloat16
f8t = ml_dtypes.float8_e4m3

PHASES = int(os.environ.get("BASS_PHASES", "5"))
DEBUG = int(os.environ.get("BASS_DEBUG", "0"))

CSPLIT = int(os.environ.get("BASS_CSPLIT", "19"))  # conv taps on PE
CSC = int(os.environ.get("BASS_CSC", "8"))         # conv taps on scalar engine
F8SC = 64.0                                        # fp8 weight scale

# Schraudolph exp on the bf16 grid: bf16bits(exp(s)) ~ s*2^7/ln2 + b
EXP_A16 = 128.0 / float(np.log(2.0))
EXP_B16 = 127.0 * 128.0 - 366392.3 / 65536.0


# ---------------------------------------------------------------- host prep

def _shuffle_w(W):
    """[Kd, Nd] -> [NC, 128, Kd] where slab n is the SBUF lhsT tile for
    output chunk n: tile[:, kc*128:(kc+1)*128] = W[kc-chunk, n-chunk]."""
    Kd, Nd = W.shape
    KC, NC = Kd // P, Nd // P
    arr = W.reshape(KC, P, NC, P).transpose(2, 1, 0, 3).reshape(NC, P, Kd)
    return np.ascontiguousarray(arr)


def _w8(W):
    """fp8 x64 k-pair slab: [Kd, Nd] -> (NC, P, KC, P) f8."""
    Kd, Nd = W.shape
    KC = Kd // P
    return np.ascontiguousarray(
        _shuffle_w(np.asarray(W, np.float32) * F8SC).reshape(-1, P, KC, P)
    ).astype(f8t)


def _cols(v):
    """[N] bias -> [128, N/128] column tile (col n = bias of chunk n)."""
    return np.ascontiguousarray(v.reshape(-1, P).T)


def prep_inputs(inputs):
    f32 = np.float32
    g = {}

    def W(name):
        return np.asarray(inputs[name], f32)

    # ff1 (LN gamma/beta folded; 0.5 residual scale folded into w2/b2)
    g['w1a'] = _w8(W('ff1_ng')[:, None] * W('ff1_w1'))
    g['c1a'] = _cols(W('ff1_nb') @ W('ff1_w1') + W('ff1_b1'))
    g['w2a'] = _w8(0.5 * W('ff1_w2'))
    g['b2a'] = (0.5 * W('ff1_b2') * F8SC).reshape(1, D)
    # attention projections (1/sqrt(HD) folded into q)
    g['wqa'] = _w8(W('attn_ng')[:, None] * W('wq') * (HD ** -0.5))
    g['cqa'] = _cols((W('attn_nb') @ W('wq')) * (HD ** -0.5))
    g['wkvaa'] = _w8((W('attn_ng')[:, None] * W('wkva'))[:, :R])
    g['ckvaa'] = _cols((W('attn_nb') @ W('wkva'))[:R])
    g['wkvba'] = _shuffle_w(W('kvn_g')[:, None] * W('wkvb')).astype(bf16)
    g['ckvba'] = _cols(W('kvn_b') @ W('wkvb'))
    g['woa'] = _w8(W('wo'))
    # conv module
    g['wp1a'] = _w8(W('conv_ng')[:, None] * W('pw1_w'))
    g['cp1a'] = _cols(W('conv_nb') @ W('pw1_w') + W('pw1_b'))
    sbn = W('bn_g') / np.sqrt(W('bn_rv') + EPS)
    g['tbna'] = _cols((W('dw_b') - W('bn_rm')) * sbn + W('bn_b'))
    g['wp2a'] = _w8(W('pw2_w'))
    g['bp2a'] = (W('pw2_b') * F8SC).reshape(1, D)
    dwf = np.asarray(inputs['dw_w'], f32)[:, 0, :] * sbn[:, None]  # [D, 31]
    g['dwcol'] = np.ascontiguousarray(
        dwf.reshape(DC, P, KW).transpose(1, 0, 2).reshape(P, DC * KW))
    diag = np.zeros((DC, P, CSPLIT, P), f32)
    idx = np.arange(P)
    for c in range(DC):
        for j in range(CSPLIT):
            diag[c, idx, j, idx] = dwf[c * P:(c + 1) * P, j]
    g['diaga'] = diag.reshape(DC, P, CSPLIT * P).astype(bf16)
    # ff2
    g['w1b'] = _w8(W('ff2_ng')[:, None] * W('ff2_w1'))
    g['c1b'] = _cols(W('ff2_nb') @ W('ff2_w1') + W('ff2_b1'))
    g['w2b'] = _w8(0.5 * W('ff2_w2'))
    g['b2b'] = (0.5 * W('ff2_b2') * F8SC).reshape(1, D)
    # final LN affine
    g['finga'] = _cols(W('fin_g'))
    g['finba'] = _cols(W('fin_b'))
    # rope tables (transposed, tiled x2 heads per 128 partitions)
    inv = 1.0 / (10000.0 ** (np.arange(0, HD, 2, dtype=f32) / HD))
    t = np.arange(T, dtype=f32)
    fr = np.einsum('i,j->ij', t, inv)
    emb = np.concatenate([fr, fr], -1)                        # [T, 64]
    cosT = np.cos(emb).T.astype(f32)                          # [64, T]
    sinT = np.sin(emb).T.astype(f32)
    g['cos2'] = np.ascontiguousarray(
        np.concatenate([cosT, cosT], 0)).astype(bf16)
    g['sin2'] = np.ascontiguousarray(
        np.concatenate([sinT, sinT], 0)).astype(bf16)
    p2 = np.zeros((P, P), f32)
    for b in range(2):
        o = 64 * b
        for d_ in range(32):
            p2[o + 32 + d_, o + d_] = -1.0
            p2[o + d_, o + 32 + d_] = 1.0
    g['p2m'] = p2.astype(bf16)
    id2 = np.zeros((P, P), f32)
    id2[0:64, 0:64] = np.eye(64, dtype=f32)
    id2[64:P, 0:64] = np.eye(64, dtype=f32)
    g['ident'] = id2
    g['ones1'] = np.ones((1, P), f32)
    sel2 = np.zeros((2, P), f32)
    sel2[0, 0:64] = 1.0
    sel2[1, 64:P] = 1.0
    g['sel2'] = sel2
    g['onesp'] = np.ones((P, 1), f32)
    g['onespb'] = np.ones((P, 1), f32).astype(bf16)
    g['onesrow'] = np.ones((1, T), f32)
    return g


# ------------------------------------------------------------- device build

def build():
    nc = bacc.Bacc("TRN2", target_bir_lowering=False, debug=False,
                   enable_asserts=False, num_devices=N_CORES)
    f32, f32r, b16, f8, i16 = (dt.float32, dt.float32r, dt.bfloat16,
                               dt.float8e4, dt.int16)

    def din(name, shape, d):
        return nc.dram_tensor(name, shape, d, kind="ExternalInput").ap()

    xT = din('xT', (D, T), f32r)
    w1a = din('w1a', (FFC, P, DC, P), f8)
    c1a = din('c1a', (P, FFC), f32)
    w2a = din('w2a', (DC, P, FFC, P), f8)
    b2a = din('b2a', (1, D), f32r)
    wqa = din('wqa', (8, P, DC, P), f8)
    cqa = din('cqa', (P, 8), f32)
    wkvaa = din('wkvaa', (RC, P, DC, P), f8)
    ckvaa = din('ckvaa', (P, RC), f32)
    wkvba = din('wkvba', (4, P, R), b16)
    ckvba = din('ckvba', (P, 4), f32)
    woa = din('woa', (DC, P, DC, P), f8)
    wp1a = din('wp1a', (16, P, DC, P), f8)
    cp1a = din('cp1a', (P, 16), f32)
    tbna = din('tbna', (P, DC), f32)
    wp2a = din('wp2a', (DC, P, DC, P), f8)
    bp2a = din('bp2a', (1, D), f32r)
    dwcold = din('dwcol', (P, DC * KW), f32)
    diaga = din('diaga', (DC, P, CSPLIT * P), b16)
    w1b = din('w1b', (FFC, P, DC, P), f8)
    c1b = din('c1b', (P, FFC), f32)
    w2b = din('w2b', (DC, P, FFC, P), f8)
    b2b = din('b2b', (1, D), f32r)
    finga = din('finga', (P, DC), f32)
    finba = din('finba', (P, DC), f32)
    cos2d = din('cos2', (P, T), b16)
    sin2d = din('sin2', (P, T), b16)
    p2md = din('p2m', (P, P), b16)
    identd = din('ident', (P, P), f32r)
    ones1d = din('ones1', (1, P), f32r)
    sel2d = din('sel2', (2, P), f32r)
    onespd = din('onesp', (P, 1), f32r)
    onespbd = din('onespb', (P, 1), b16)
    onesrowd = din('onesrow', (1, T), f32r)

    outT = nc.dram_tensor('outT', (D, T), f32r, kind="ExternalOutput").ap()

    def ddram(name, shape, d):
        return nc.dram_tensor(name, shape, d, kind="ExternalOutput").ap()

    with tile.TileContext(nc) as tc, ExitStack() as top:
        cpool = top.enter_context(tc.tile_pool(name="const", bufs=1))
        res_pool = top.enter_context(tc.tile_pool(name="res", bufs=1))
        xh_pool = top.enter_context(tc.tile_pool(name="xh", bufs=1))

        res = []
        for c in range(DC):
            r_ = res_pool.tile([P, T], f32r, name=f"res{c}")
            nc.sync.dma_start(r_[:], xT[c * P:(c + 1) * P, :])
            res.append(r_)

        def ctile(src, shape, d, name):
            t_ = cpool.tile(shape, d, name=name)
            nc.sync.dma_start(t_[:], src[:])
            return t_

        c1t = ctile(c1a, [P, FFC], f32, "c1t")
        b2at = ctile(b2a, [1, D], f32r, "b2at")
        cqt = ctile(cqa, [P, 8], f32, "cqt")
        ckvat = ctile(ckvaa, [P, RC], f32, "ckvat")
        ckvbt = ctile(ckvba, [P, 4], f32, "ckvbt")
        cp1t = ctile(cp1a, [P, 16], f32, "cp1t")
        tbnt = ctile(tbna, [P, DC], f32, "tbnt")
        bp2t = ctile(bp2a, [1, D], f32r, "bp2t")
        c1bt = ctile(c1b, [P, FFC], f32, "c1bt")
        b2bt = ctile(b2b, [1, D], f32r, "b2bt")
        fingt = ctile(finga, [P, DC], f32, "fingt")
        finbt = ctile(finba, [P, DC], f32, "finbt")
        cos2t = ctile(cos2d, [P, T], b16, "cos2t")
        sin2t = ctile(sin2d, [P, T], b16, "sin2t")
        p2mt = ctile(p2md, [P, P], b16, "p2mt")
        identt = ctile(identd, [P, P], f32r, "identt")
        ones1t = ctile(ones1d, [1, P], f32r, "ones1t")
        sel2t = ctile(sel2d, [2, P], f32r, "sel2t")
        onespt = ctile(onespd, [P, 1], f32r, "onespt")
        onespbt = ctile(onespbd, [P, 1], b16, "onespbt")
        onesrt = ctile(onesrowd, [1, T], f32r, "onesrt")
        dwcolt = ctile(dwcold, [P, DC * KW], f32, "dwcolt")
        epst = cpool.tile([P, 1], dt.float32, name="epst")
        nc.gpsimd.memset(epst[:], EPS)

        # -------- layernorm stats machinery (emitted inside prev phase) ----
        def make_stats(tag, dred):
            """Returns (after_dch, finish). after_dch(dch) accumulates
            square/sum stats for res[dch] into a single [33,T] PSUM tile
            (s1 row 0, s2 row 32 - the two ones-matmuls col-pack); pools
            open lazily at the first call. finish() runs the serial chain,
            emits (A_b, NMA_b) [128,T] broadcast tiles through a transient
            2-bank-pair PSUM pool, then frees all stats resources."""
            sctx = ExitStack()
            st = {}

            def after_dch(dch):
                if 'stat' not in st:
                    lnp = sctx.enter_context(
                        tc.tile_pool(name=f"lnp_{tag}", bufs=1, space="PSUM",
                                     side="right"))
                    st['lns'] = sctx.enter_context(
                        tc.tile_pool(name=f"lns_{tag}", bufs=1, side="right"))
                    st['stat'] = lnp.tile([33, T], f32, name=f"st_{tag}")
                stat, lns = st['stat'], st['lns']
                s_ = lns.tile([P, T], b16, tag="sq", bufs=2,
                              name=f"sq_{tag}{dch}")
                nc.scalar.square(s_[:], res[dch].bitcast(f32)[:])
                for h in range(2):
                    sl = slice(h * 512, (h + 1) * 512)
                    nc.tensor.matmul(stat[0:1, sl], onespt[:],
                                     res[dch][:, sl],
                                     start=(dch == 0), stop=(dch == DC - 1))
                    nc.tensor.matmul(stat[32:33, sl], onespbt[:], s_[:, sl],
                                     start=(dch == 0), stop=(dch == DC - 1))

            def finish():
                stat, lns = st['stat'], st['lns']
                m_t = lns.tile([1, T], f32r, name=f"m_{tag}")
                nc.vector.tensor_scalar(m_t[:], stat[0:1, :], 1.0 / dred,
                                        None, Alu.mult)
                ms = lns.tile([1, T], f32, name=f"ms_{tag}")
                nc.scalar.square(ms[:], m_t.bitcast(f32)[:])
                v_ = lns.tile([1, T], f32, name=f"v_{tag}")
                nc.vector.scalar_tensor_tensor(v_[:], stat[32:33, :],
                                               1.0 / dred, ms[:],
                                               Alu.mult, Alu.subtract)
                sd = lns.tile([1, T], f32, name=f"sd_{tag}")
                nc.scalar.activation(sd[:], v_[:], Act.Sqrt,
                                     bias=epst[0:1, 0:1])
                af = lns.tile([1, T], f32, name=f"af_{tag}")
                nc.vector.reciprocal_approx_fast(out=af[:], in_=sd[:])
                a_t = lns.tile([1, T], f32r, name=f"a_{tag}")
                nc.vector.tensor_copy(a_t[:], af[:])
                nma_t = lns.tile([1, T], f32r, name=f"nma_{tag}")
                nc.vector.scalar_tensor_tensor(nma_t[:], m_t.bitcast(f32)[:],
                                               -1.0, af[:], Alu.mult,
                                               Alu.mult)
                ab_s = xh_pool.tile([P, T], f32, tag="stA", name=f"abs_{tag}")
                nmb_s = xh_pool.tile([P, T], f32, tag="stM",
                                     name=f"nmbs_{tag}")
                with tc.tile_pool(name=f"bc_{tag}", bufs=2,
                                  space="PSUM") as bp:
                    abp = bp.tile([P, T], f32, tag="bc", name=f"abp_{tag}")
                    nmp = bp.tile([P, T], f32, tag="bc", name=f"nmp_{tag}")
                    for h in range(2):
                        sl = slice(h * 512, (h + 1) * 512)
                        nc.tensor.matmul(abp[:, sl], ones1t[:], a_t[:, sl],
                                         start=True, stop=True)
                        nc.tensor.matmul(nmp[:, sl], ones1t[:], nma_t[:, sl],
                                         start=True, stop=True)
                    nc.scalar.copy(ab_s[:], abp[:])
                    nc.scalar.copy(nmb_s[:], nmp[:])
                sctx.close()
                return ab_s, nmb_s

            return after_dch, finish

        def ln_std8(tag, finish):
            """Standardize res into fp8 pair-packed xq tiles [P,2,T]
            (pair j = chunks 2j, 2j+1), emitted per-chunk so downstream
            k-loops start as soon as their pair lands."""
            ab_s, nmb_s = finish()
            xq = [xh_pool.tile([P, 2, T], dt.float8e4, tag=f"xq{j}",
                               name=f"xq_{tag}{j}") for j in range(DC // 2)]
            with tc.tile_pool(name=f"lnu_{tag}", bufs=1) as lnu:
                for c in range(DC):
                    tm = lnu.tile([P, T], f32, tag="lnt", bufs=2,
                                  name=f"lnt_{tag}{c}")
                    nc.vector.tensor_tensor(tm[:], res[c].bitcast(f32)[:],
                                            ab_s[:], Alu.mult)
                    nc.vector.tensor_tensor(xq[c // 2][:, c % 2, :], tm[:],
                                            nmb_s[:], Alu.add)
            return xq

        # -------- fp8 DoubleRow projection: out chunk via k-pair loop -----
        def mmproj8(pool, wt, xq, kc2, nm, evict, brow=None):
            ps = pool.tile([P, 1024], dt.float32, tag="mm", name=nm)
            for k in range(kc2):
                w_ = wt[:, 2 * k:2 * k + 2, :]
                for h in range(2):
                    nc.tensor.matmul(ps[:, h * 512:(h + 1) * 512], w_,
                                     xq[k][:, :, h * 512:(h + 1) * 512],
                                     start=(k == 0),
                                     stop=(k == kc2 - 1 and brow is None),
                                     perf_mode=DR)
            if brow is not None:
                for h in range(2):
                    sl = slice(h * 512, (h + 1) * 512)
                    nc.tensor.matmul(ps[:, sl], brow, onesrt[:, sl],
                                     start=False, stop=(h == 1))
            evict(ps)

        # ---------------- feed-forward macaron ----------------
        def ffn(tag, w1d, c1tile, w2d, browt, finish_self, after_dch=None):
            xq = ln_std8(tag, finish_self)
            with ExitStack() as ctx:
                wp = ctx.enter_context(tc.tile_pool(name=f"w_{tag}", bufs=3))
                hp = ctx.enter_context(tc.tile_pool(name=f"h1_{tag}", bufs=1))
                pp = ctx.enter_context(
                    tc.tile_pool(name=f"ps_{tag}", bufs=2, space="PSUM"))
                h1q = [hp.tile([P, 2, T], dt.float8e4, tag=f"h1q{j}",
                               name=f"h1q_{tag}{j}") for j in range(FFC // 2)]
                for n in range(FFC):
                    wt = wp.tile([P, DC, P], dt.float8e4, tag="w1",
                                 name=f"w1_{tag}{n}")
                    nc.sync.dma_start(wt[:], w1d[n])

                    def ev1(ps, n=n):
                        nc.scalar.activation(h1q[n // 2][:, n % 2, :], ps[:],
                                             Act.Silu,
                                             bias=c1tile[:, n:n + 1],
                                             scale=1.0 / F8SC)
                    mmproj8(pp, wt, xq, DC // 2, f"p1_{tag}{n}", ev1)
                for dch in range(DC):
                    wt2 = wp.tile([P, FFC, P], dt.float8e4, tag="w2", bufs=2,
                                  name=f"w2_{tag}{dch}")
                    nc.sync.dma_start(wt2[:], w2d[dch])

                    def ev2(ps, dch=dch):
                        nc.vector.scalar_tensor_tensor(
                            res[dch][:], ps[:], 1.0 / F8SC,
                            res[dch].bitcast(f32)[:], Alu.mult, Alu.add)
                    mmproj8(pp, wt2, h1q, FFC // 2, f"p2_{tag}{dch}", ev2,
                            brow=browt[0:1, dch * P:(dch + 1) * P])
                    if after_dch is not None:
                        after_dch(dch)

        # ---------------- kv-latent layernorm (bf16 out) ----------------
        def ln_small(tag, src_tiles, nch, dred, bpool):
            """Standardize bf16 src over nch*128 channels; mb/ab broadcast
            pairs borrow bpool's "mm" tag (no extra PSUM banks)."""
            with ExitStack() as ctx:
                lnp = ctx.enter_context(
                    tc.tile_pool(name=f"lnp_{tag}", bufs=1, space="PSUM"))
                lns = ctx.enter_context(
                    tc.tile_pool(name=f"lns_{tag}", bufs=1))
                stat = lnp.tile([33, T], f32, name=f"st_{tag}")
                sq = []
                for c in range(nch):
                    s_ = lns.tile([P, T], b16, tag="sq", bufs=2,
                                  name=f"sq_{tag}{c}")
                    nc.scalar.square(s_[:], src_tiles[c][:])
                    sq.append(s_)
                for c in range(nch):
                    for h in range(2):
                        sl = slice(h * 512, (h + 1) * 512)
                        nc.tensor.matmul(stat[0:1, sl], onespbt[:],
                                         src_tiles[c][:, sl],
                                         start=(c == 0), stop=(c == nch - 1))
                        nc.tensor.matmul(stat[32:33, sl], onespbt[:],
                                         sq[c][:, sl],
                                         start=(c == 0), stop=(c == nch - 1))
                m_t = lns.tile([1, T], f32r, name=f"m_{tag}")
                nc.vector.tensor_scalar(m_t[:], stat[0:1, :], 1.0 / dred,
                                        None, Alu.mult)
                ms = lns.tile([1, T], f32, name=f"ms_{tag}")
                nc.scalar.square(ms[:], m_t.bitcast(f32)[:])
                v_ = lns.tile([1, T], f32, name=f"v_{tag}")
                nc.vector.scalar_tensor_tensor(v_[:], stat[32:33, :],
                                               1.0 / dred, ms[:],
                                               Alu.mult, Alu.subtract)
                sd = lns.tile([1, T], f32, name=f"sd_{tag}")
                nc.scalar.activation(sd[:], v_[:], Act.Sqrt,
                                     bias=epst[0:1, 0:1])
                af = lns.tile([1, T], f32, name=f"af_{tag}")
                nc.vector.reciprocal_approx_fast(out=af[:], in_=sd[:])
                a_t = lns.tile([1, T], f32r, name=f"a_{tag}")
                nc.vector.tensor_copy(a_t[:], af[:])
                mb = bpool.tile([P, 1024], f32, tag="mm", name=f"mb_{tag}")
                ab = bpool.tile([P, 1024], f32, tag="mm", name=f"ab_{tag}")
                for h in range(2):
                    sl = slice(h * 512, (h + 1) * 512)
                    nc.tensor.matmul(mb[:, sl], ones1t[:], m_t[:, sl],
                                     start=True, stop=True)
                    nc.tensor.matmul(ab[:, sl], ones1t[:], a_t[:, sl],
                                     start=True, stop=True)
                outs = []
                for c in range(nch):
                    t_ = xh_pool.tile([P, T], b16, tag=f"lat{c}",
                                      name=f"lat_{tag}{c}")
                    tm = lns.tile([P, T], f32, tag="lntmp", bufs=2,
                                  name=f"lntmp_{tag}{c}")
                    nc.vector.tensor_tensor(tm[:], src_tiles[c][:], mb[:],
                                            Alu.subtract)
                    nc.vector.tensor_tensor(t_[:], tm[:], ab[:], Alu.mult)
                    outs.append(t_)
            return outs

        # ---------------- attention ----------------
        def attn(finish_at, conv_after_dch):
            xq = ln_std8("at", finish_at)
            with ExitStack() as ctx:
                wp = ctx.enter_context(tc.tile_pool(name="w_at", bufs=2))
                kv_pool = ctx.enter_context(tc.tile_pool(name="kvt", bufs=1))
                pre_ctx = ExitStack()
                pre = pre_ctx.enter_context(
                    tc.tile_pool(name="pre_at", bufs=1))

                qpre, kva = [], []
                with tc.tile_pool(name="pA", bufs=2, space="PSUM") as pA:
                    # kva projection first: the kv-latent LN chain then
                    # overlaps the q-projection PE bursts
                    for n in range(RC):
                        wt = wp.tile([P, DC, P], dt.float8e4, tag="w1",
                                     name=f"wkva{n}")
                        nc.sync.dma_start(wt[:], wkvaa[n])
                        kv_ = pre.tile([P, T], b16, tag=f"kva{n}",
                                       name=f"kva{n}")

                        def evkva(ps, kv_=kv_, n=n):
                            nc.scalar.activation(kv_[:], ps[:], Act.Identity,
                                                 bias=ckvat[:, n:n + 1],
                                                 scale=1.0 / F8SC)
                        mmproj8(pA, wt, xq, DC // 2, f"pkva{n}", evkva)
                        kva.append(kv_)
                    for n in range(8):
                        wt = wp.tile([P, DC, P], dt.float8e4, tag="w1",
                                     name=f"wq{n}")
                        nc.sync.dma_start(wt[:], wqa[n])
                        q_ = kv_pool.tile([P, T], b16, tag=f"q{n}",
                                          name=f"qpre{n}")

                        def evq(ps, q_=q_, n=n):
                            nc.scalar.activation(q_[:], ps[:], Act.Identity,
                                                 bias=cqt[:, n:n + 1],
                                                 scale=1.0 / F8SC)
                        mmproj8(pA, wt, xq, DC // 2, f"pq{n}", evq)
                        qpre.append(q_)
                    # latent LN (broadcasts borrow pA's mm buffers)
                    lat = ln_small("kv", kva, RC, R, pA)
                    # kvb projection (bf16): rows 0..255 = k, 256..511 = v
                    kpre, vtt = [], []
                    for n in range(4):
                        wt = wp.tile([P, R], b16, tag="wkvb",
                                     name=f"wkvb{n}")
                        nc.sync.dma_start(wt[:], wkvba[n])
                        kv_ = pre.tile([P, T], b16 if n < 2 else f32r,
                                       tag=f"kvb{n}", name=f"kvb{n}")
                        ps = pA.tile([P, 1024], f32, tag="mm",
                                     name=f"pkvb{n}")
                        for k in range(RC):
                            for h in range(2):
                                sl = slice(h * 512, (h + 1) * 512)
                                nc.tensor.matmul(ps[:, sl],
                                                 wt[:, k * P:(k + 1) * P],
                                                 lat[k][:, sl],
                                                 start=(k == 0),
                                                 stop=(k == RC - 1))
                        nc.scalar.activation(kv_[:], ps[:], Act.Identity,
                                             bias=ckvbt[:, n:n + 1])
                        (kpre if n < 2 else vtt).append(kv_)
                # v: transpose to token-major + ones col -> v_aug (bf16)
                vaug = []
                for c in range(DC):
                    va = kv_pool.tile([P, KVH * 65], b16, tag=f"va{c}",
                                      name=f"vaug{c}")
                    for g_ in range(KVH):
                        nc.gpsimd.memset(
                            va[:, g_ * 65 + 64:g_ * 65 + 65], 1.0)
                    vaug.append(va)
                with tc.tile_pool(name="pV", bufs=4, space="PSUM") as pV:
                    for g_ in range(KVH):
                        src = vtt[g_ // 2]
                        off = 64 * (g_ % 2)
                        for c in range(DC):
                            pt_ = pV.tile([P, 64], f32r, tag="vt",
                                          name=f"vt{g_}_{c}")
                            nc.tensor.matmul(pt_[:],
                                             src[off:off + 64,
                                                 c * P:(c + 1) * P],
                                             identt[off:off + 64, 0:64],
                                             is_transpose=True,
                                             start=True, stop=True)
                            nc.scalar.copy(
                                vaug[c][:, g_ * 65:g_ * 65 + 64],
                                pt_.bitcast(f32)[:])

                # rope on k first (scores for early head-pairs can start
                # before the q ropes finish), then q -> bf16
                roped = []
                with tc.tile_pool(name="pR", bufs=2, space="PSUM") as pR, \
                        tc.tile_pool(name="rtp", bufs=1) as rtp:
                    for i, src in enumerate(kpre + qpre):
                        pq = pR.tile([P, T], f32, tag="rope",
                                     name=f"ropep{i}")
                        for h in range(2):
                            sl = slice(h * 512, (h + 1) * 512)
                            nc.tensor.matmul(pq[:, sl], p2mt[:], src[:, sl],
                                             start=True, stop=True)
                        pqs = rtp.tile([P, T], b16, tag="pqs", bufs=2,
                                       name=f"pqs{i}")
                        nc.scalar.copy(pqs[:], pq[:])
                        t1 = rtp.tile([P, T], b16, tag="ropet1", bufs=2,
                                      name=f"ropet1_{i}")
                        nc.vector.tensor_tensor(t1[:], src[:], cos2t[:],
                                                Alu.mult)
                        t2 = rtp.tile([P, T], b16, tag="ropet2", bufs=2,
                                      name=f"ropet2_{i}")
                        nc.vector.tensor_tensor(t2[:], pqs[:], sin2t[:],
                                                Alu.mult)
                        r_ = (pre if i < 2 else kv_pool).tile(
                            [P, T], b16,
                            tag=(f"kro{i}" if i < 2 else f"q{i - 2}"),
                            name=f"roped{i}")
                        nc.vector.tensor_tensor(r_[:], t1[:], t2[:], Alu.add)
                        roped.append(r_)
                    krc = roped[:2]
                    kr2 = []
                    for g_ in range(KVH):
                        k2 = kv_pool.tile([P, T], b16, tag=f"kr2_{g_}",
                                          name=f"kr2_{g_}")
                        off = 64 * (g_ % 2)
                        src = krc[g_ // 2]
                        nc.vector.tensor_copy(k2[0:64, :],
                                              src[off:off + 64, :])
                        nc.vector.tensor_copy(k2[64:P, :],
                                              src[off:off + 64, :])
                        kr2.append(k2)
                qr = roped[2:]
                pre_ctx.close()

                # scores -> exp -> pT ; oT via v_aug (denominator in row
                # 64); per-head-pair softmax normalization. exp split
                # between scalar (ACT Exp) and vector (Schraudolph int16
                # bit-trick, bitcast bf16). Software-pipelined: attnV of
                # head-pair hp-1 is emitted after the scores of hp, so the
                # PE never waits on the exp evicts.
                otq = [kv_pool.tile([P, 2, T], dt.float8e4, tag=f"otq{j}",
                                    name=f"otq{j}") for j in range(DC // 2)]
                with ExitStack() as sctx:
                    scp = sctx.enter_context(
                        tc.tile_pool(name="scp", bufs=2, space="PSUM"))
                    otp = sctx.enter_context(
                        tc.tile_pool(name="otp", bufs=2, space="PSUM"))
                    ptp = sctx.enter_context(tc.tile_pool(name="ptp",
                                                          bufs=2))
                    otup = sctx.enter_context(
                        tc.tile_pool(name="otup", bufs=1))
                    ptq = [None] * 9

                    def scores(hp):
                        kt = kr2[hp // 2]
                        qt = qr[hp]
                        pt = [[None] * DC for _ in range(2)]
                        for c in range(DC):
                            ps2 = [scp.tile([P, 1024], f32, tag="sc",
                                            name=f"sc{hp}_{c}{s}")
                                   for s in range(2)]
                            for th in range(2):
                                sl = slice(th * 512, (th + 1) * 512)
                                for sub in range(2):
                                    qo = 64 * sub
                                    nc.tensor.matmul(
                                        ps2[sub][:, sl],
                                        kt[qo:qo + 64, c * P:(c + 1) * P],
                                        qt[qo:qo + 64, sl],
                                        start=True, stop=True)
                            for sub in range(2):
                                if (c + sub) % 2 == 0:
                                    p_ = ptp.tile([P, T], b16,
                                                  tag=f"pt{sub}_{c}",
                                                  name=f"pt{hp}_{c}{sub}")
                                    nc.scalar.activation(p_[:], ps2[sub][:],
                                                         Act.Exp)
                                else:
                                    pi = ptp.tile([P, T], i16,
                                                  tag=f"pt{sub}_{c}",
                                                  name=f"pti{hp}_{c}{sub}")
                                    nc.vector.tensor_scalar(
                                        pi[:], ps2[sub][:], EXP_A16,
                                        EXP_B16, Alu.mult, Alu.add)
                                    p_ = pi.bitcast(b16)
                                pt[sub][c] = p_
                        return pt

                    def attnv(hp):
                        g_ = hp // 2
                        pt = ptq[hp]
                        ous = []
                        for sub in range(2):
                            pos = otp.tile([65, 1024], f32, tag="ot",
                                           name=f"ot{hp}_{sub}")
                            for c in range(DC):
                                for th in range(2):
                                    sl = slice(th * 512, (th + 1) * 512)
                                    nc.tensor.matmul(
                                        pos[:, sl],
                                        vaug[c][:, g_ * 65:(g_ + 1) * 65],
                                        pt[sub][c][:, sl],
                                        start=(c == 0), stop=(c == DC - 1))
                            ou = otup.tile([65, T], f32, tag=f"otu{sub}",
                                           bufs=1, name=f"otu{hp}_{sub}")
                            nc.scalar.copy(ou[:], pos[:])
                            ous.append(ou)
                        den2 = otup.tile([2, T], f32, tag="den", bufs=1,
                                         name=f"den{hp}")
                        nc.sync.dma_start(den2[0:1, :], ous[0][64:65, :])
                        nc.sync.dma_start(den2[1:2, :], ous[1][64:65, :])
                        recf2 = otup.tile([2, T], f32, tag="recf", bufs=1,
                                          name=f"recf{hp}")
                        nc.vector.reciprocal_approx_fast(out=recf2[:],
                                                         in_=den2[:])
                        recr2 = otup.tile([2, T], f32r, tag="recr", bufs=1,
                                          name=f"recr{hp}")
                        nc.vector.tensor_copy(recr2[:], recf2[:])
                        rb = otp.tile([P, 1024], f32, tag="ot",
                                      name=f"rb{hp}")
                        for th in range(2):
                            sl = slice(th * 512, (th + 1) * 512)
                            nc.tensor.matmul(rb[:, sl], sel2t[:],
                                             recr2[:, sl],
                                             start=True, stop=True)
                        for sub in range(2):
                            nc.vector.tensor_tensor(
                                otq[hp // 2][sub * 64:(sub + 1) * 64,
                                             hp % 2, :],
                                ous[sub][0:64, :],
                                rb[sub * 64:(sub + 1) * 64, :], Alu.mult)

                    for hp in range(9):
                        if hp < 8:
                            ptq[hp] = scores(hp)
                        if hp > 0:
                            attnv(hp - 1)
                # output projection + residual (+ conv LN stats)
                with tc.tile_pool(name="pO", bufs=2, space="PSUM") as pO:
                    for dch in range(DC):
                        wt = wp.tile([P, DC, P], dt.float8e4, tag="w1",
                                     name=f"wo{dch}")
                        nc.sync.dma_start(wt[:], woa[dch])

                        def evo(ps, dch=dch):
                            nc.vector.scalar_tensor_tensor(
                                res[dch][:], ps[:], 1.0 / F8SC,
                                res[dch].bitcast(f32)[:], Alu.mult, Alu.add)
                        mmproj8(pO, wt, otq, DC // 2, f"po{dch}", evo)
                        conv_after_dch(dch)

        # ---------------- conv module ----------------
        def convmod(finish_cv, ff2_after_dch):
            xq = ln_std8("cv", finish_cv)
            with ExitStack() as ctx:
                wp = ctx.enter_context(tc.tile_pool(name="w_cv", bufs=3))
                ap_ = ctx.enter_context(tc.tile_pool(name="a_cv", bufs=1))
                pp = ctx.enter_context(
                    tc.tile_pool(name="ps_cv", bufs=2, space="PSUM"))
                # pw1 "a" half lands directly in the padded glu tiles
                # (16 zeros left/right); sigmoid half in sg tiles; then GLU
                # is an in-place multiply.
                glu, sg = [], []
                for c in range(DC):
                    gp = ap_.tile([P, T + 32], b16, tag=f"glu{c}",
                                  name=f"glu{c}")
                    nc.gpsimd.memset(gp[:, 0:16], 0.0)
                    nc.gpsimd.memset(gp[:, T + 16:T + 32], 0.0)
                    glu.append(gp)
                for n in range(16):
                    wt = wp.tile([P, DC, P], dt.float8e4, tag="w1",
                                 name=f"wp1_{n}")
                    nc.sync.dma_start(wt[:], wp1a[n])
                    if n < 8:
                        o_ = glu[n]

                        def evc(ps, o_=o_, n=n):
                            nc.scalar.activation(o_[:, 16:T + 16], ps[:],
                                                 Act.Identity,
                                                 bias=cp1t[:, n:n + 1],
                                                 scale=1.0 / F8SC)
                    else:
                        o_ = ap_.tile([P, T], b16, tag=f"sg{n}",
                                      name=f"sg{n}")
                        sg.append(o_)

                        def evc(ps, o_=o_, n=n):
                            nc.scalar.activation(o_[:], ps[:], Act.Sigmoid,
                                                 bias=cp1t[:, n:n + 1],
                                                 scale=1.0 / F8SC)
                    mmproj8(pp, wt, xq, DC // 2, f"pp1_{n}", evc)
                for c in range(DC):
                    nc.vector.tensor_tensor(glu[c][:, 16:T + 16],
                                            glu[c][:, 16:T + 16], sg[c][:],
                                            Alu.mult)
                if DEBUG:
                    nc.sync.dma_start(ddram('d_glu', (P, T + 32), b16)[:],
                                      glu[0][:])
                # conv taps: out[t] += dwf[:, j] * pad[t + j + 1]
                cvq = [ap_.tile([P, 2, T], dt.float8e4, tag=f"cvq{j}",
                                name=f"cvq{j}") for j in range(DC // 2)]
                for c in range(DC):
                    wt = wp.tile([P, CSPLIT * P], b16, tag="diag", bufs=2,
                                 name=f"dg{c}")
                    nc.sync.dma_start(wt[:], diaga[c])

                    def tapsrc(j, c=c):
                        return glu[c][:, j + 1:j + 1 + T]

                    def wcol(j, c=c):
                        return dwcolt[:, c * KW + j:c * KW + j + 1]

                    # scalar-engine taps: per-channel multiply via ACT scale
                    tmps = []
                    for j in range(CSPLIT, CSPLIT + CSC):
                        tm_ = ap_.tile([P, T], b16, tag="ctmp", bufs=CSC + 2,
                                       name=f"ctmp{c}_{j}")
                        nc.scalar.activation(tm_[:], tapsrc(j), Act.Identity,
                                             scale=wcol(j))
                        tmps.append(tm_)
                    accv = ap_.tile([P, T], b16, tag="caccv", bufs=2,
                                    name=f"caccv{c}")
                    j0 = CSPLIT + CSC
                    nc.vector.tensor_scalar(accv[:], tapsrc(j0), wcol(j0),
                                            None, Alu.mult)
                    for j in range(j0 + 1, KW):
                        nc.vector.scalar_tensor_tensor(
                            accv[:], tapsrc(j), wcol(j), accv[:],
                            Alu.mult, Alu.add)
                    for tm_ in tmps:
                        nc.vector.tensor_tensor(accv[:], accv[:], tm_[:],
                                                Alu.add)
                    psc = pp.tile([P, 1024], f32, tag="mm", name=f"pcv{c}")
                    for j in range(CSPLIT):
                        for th in range(2):
                            o = th * 512 + j + 1
                            nc.tensor.matmul(
                                psc[:, th * 512:(th + 1) * 512],
                                wt[:, j * P:(j + 1) * P],
                                glu[c][:, o:o + 512],
                                start=(j == 0), stop=(j == CSPLIT - 1))
                    # fold the vector-side accumulation into the psum pair,
                    # then silu straight to fp8 pairs
                    nc.vector.tensor_tensor(psc[:], psc[:], accv[:], Alu.add)
                    nc.scalar.activation(cvq[c // 2][:, c % 2, :], psc[:],
                                         Act.Silu, bias=tbnt[:, c:c + 1])
                for dch in range(DC):
                    wt = wp.tile([P, DC, P], dt.float8e4, tag="wp2", bufs=2,
                                 name=f"wp2_{dch}")
                    nc.sync.dma_start(wt[:], wp2a[dch])

                    def evp2(ps, dch=dch):
                        nc.vector.scalar_tensor_tensor(
                            res[dch][:], ps[:], 1.0 / F8SC,
                            res[dch].bitcast(f32)[:], Alu.mult, Alu.add)
                    mmproj8(pp, wt, cvq, DC // 2, f"pp2_{dch}", evp2,
                            brow=bp2t[0:1, dch * P:(dch + 1) * P])
                    ff2_after_dch(dch)

        # ---------------- final LN (with affine) ----------------
        def make_final():
            sctx = ExitStack()
            st = {}

            def after_dch(dch):
                if 'stat' not in st:
                    lnp = sctx.enter_context(
                        tc.tile_pool(name="lnp_fin", bufs=1, space="PSUM",
                                     side="right"))
                    st['lns'] = sctx.enter_context(
                        tc.tile_pool(name="lns_fin", bufs=1, side="right"))
                    st['stat'] = lnp.tile([33, T], f32, name="st_fin")
                stat, lns = st['stat'], st['lns']
                s_ = lns.tile([P, T], b16, tag="sqf", bufs=2,
                              name=f"sq_fin{dch}")
                nc.scalar.square(s_[:], res[dch].bitcast(f32)[:])
                for h in range(2):
                    sl = slice(h * 512, (h + 1) * 512)
                    nc.tensor.matmul(stat[0:1, sl], onespt[:],
                                     res[dch][:, sl],
                                     start=(dch == 0), stop=(dch == DC - 1))
                    nc.tensor.matmul(stat[32:33, sl], onespbt[:], s_[:, sl],
                                     start=(dch == 0), stop=(dch == DC - 1))

            def finish():
                stat, lns = st['stat'], st['lns']
                with ExitStack() as fctx:
                    outp = fctx.enter_context(
                        tc.tile_pool(name="outp", bufs=2))
                    bp = fctx.enter_context(
                        tc.tile_pool(name="bc_fin", bufs=2, space="PSUM"))
                    m_t = lns.tile([1, T], f32r, name="m_fin")
                    a_t = lns.tile([1, T], f32r, name="a_fin")
                    m_r, a_r = m_t[:], a_t[:]
                    nc.vector.tensor_scalar(m_r, stat[0:1, :], 1.0 / D,
                                            None, Alu.mult)
                    ms = lns.tile([1, T], f32, name="ms_fin")
                    nc.scalar.square(ms[:], m_r.bitcast(f32))
                    v_ = lns.tile([1, T], f32, name="v_fin")
                    nc.vector.scalar_tensor_tensor(v_[:], stat[32:33, :],
                                                   1.0 / D, ms[:], Alu.mult,
                                                   Alu.subtract)
                    sd = lns.tile([1, T], f32, name="sd_fin")
                    nc.scalar.activation(sd[:], v_[:], Act.Sqrt,
                                         bias=epst[0:1, 0:1])
                    af = lns.tile([1, T], f32, name="af_fin")
                    nc.vector.reciprocal_approx_fast(out=af[:], in_=sd[:])
                    nc.vector.tensor_copy(a_r, af[:])
                    mb = bp.tile([P, T], f32, tag="bc", name="mb_fin")
                    ab = bp.tile([P, T], f32, tag="bc", name="ab_fin")
                    for h in range(2):
                        sl = slice(h * 512, (h + 1) * 512)
                        nc.tensor.matmul(mb[:, sl], ones1t[:], m_r[:, sl],
                                         start=True, stop=True)
                        nc.tensor.matmul(ab[:, sl], ones1t[:], a_r[:, sl],
                                         start=True, stop=True)
                    for c in range(DC):
                        tm = lns.tile([P, T], f32, tag="lntmp", bufs=2,
                                      name=f"fintmp{c}")
                        nc.vector.tensor_tensor(tm[:], res[c].bitcast(f32),
                                                mb[:], Alu.subtract)
                        u_ = lns.tile([P, T], f32, tag="lnu", bufs=2,
                                      name=f"finu{c}")
                        nc.vector.scalar_tensor_tensor(
                            u_[:], tm[:], fingt[:, c:c + 1], ab[:],
                            Alu.mult, Alu.mult)
                        o_ = outp.tile([P, T], f32r, tag="out",
                                       name=f"out{c}")
                        nc.scalar.activation(o_[:], u_[:], Act.Identity,
                                             bias=finbt[:, c:c + 1])
                        nc.sync.dma_start(outT[c * P:(c + 1) * P, :], o_[:])
                sctx.close()
            return after_dch, finish

        # ---------------- phase sequencing ----------------
        ff1_ad, ff1_fin = make_stats("ff1", D)
        for c in range(DC):
            ff1_ad(c)
        at_ad, at_fin = make_stats("at", D)
        ffn("ff1", w1a, c1t, w2a, b2at, ff1_fin,
            after_dch=at_ad if PHASES >= 2 else None)
        if DEBUG:
            dr1 = ddram('d_res1', (D, T), f32r)
            for c in range(DC):
                nc.sync.dma_start(dr1[c * P:(c + 1) * P, :], res[c][:])
        if PHASES >= 2:
            cv_ad, cv_fin = make_stats("cv", D)
            attn(at_fin,
                 cv_ad if PHASES >= 3 else (lambda d: None))
            if DEBUG:
                dr2 = ddram('d_res2', (D, T), f32r)
                for c in range(DC):
                    nc.sync.dma_start(dr2[c * P:(c + 1) * P, :], res[c][:])
        if PHASES >= 3:
            ff2_ad, ff2_fin = make_stats("ff2", D)
            convmod(cv_fin, ff2_ad if PHASES >= 4 else (lambda d: None))
            if DEBUG:
                dr3 = ddram('d_res3', (D, T), f32r)
                for c in range(DC):
                    nc.sync.dma_start(dr3[c * P:(c + 1) * P, :], res[c][:])
        if PHASES >= 4:
            if PHASES >= 5:
                fin_ad, fin_fin = make_final()
            ffn("ff2", w1b, c1bt, w2b, b2bt, ff2_fin,
                after_dch=fin_ad if PHASES >= 5 else None)
            if PHASES >= 5:
                fin_fin()
        if PHASES < 5:
            for c in range(DC):
                nc.sync.dma_start(outT[c * P:(c + 1) * P, :], res[c][:])

    nc.compile()
    return nc


# ------------------------------------------------------------------ driver

_NC_CACHE = {}


def _get_nc():
    key = (PHASES, DEBUG)
    if key not in _NC_CACHE:
        _NC_CACHE[key] = build()
    return _NC_CACHE[key]


def kernel(**inputs):
    nc = _get_nc()
    shared = prep_inputs(inputs)
    x = np.asarray(inputs['x'], np.float32)
    in_maps = []
    for b in range(N_CORES):
        m = dict(shared)
        m['xT'] = np.ascontiguousarray(x[b].T)
        in_maps.append(m)
    res = run_bass_kernel_spmd(nc, in_maps, core_ids=list(range(N_CORES)))
    out = np.stack([np.ascontiguousarray(r['outT'].T) for r in res.results])
    kernel.last_results = res
    return out.astype(np.float32)
